# revision 27
# baseline (speedup 1.0000x reference)
"""Trainium2 Bass kernel for batched channel-attention (nn_Attention_28071906246667).

Reference computation (per batch element n, with xT = batch_flat[n] of shape [C, HW]):
    x   = xT.T                                  # [HW, C]
    Q   = x @ Wq.T + bq ; K, V likewise         # [HW, D]
    S   = Q.T @ K                               # [D, D]
    att = softmax(S, axis=-1)
    out = att @ V.T                             # [D, HW]

Key algebraic restructuring (halves FLOPs, avoids materializing Q/K/V):
    G = x.T x  (Gram over channels), m = column sums of x. Then
      S   = Wq G Wk.T + (Wq m) bk.T + bq (Wk m).T + HW bq bk.T
          = Wq_aug @ U,   U = [G m; m.T HW] @ WkT_aug
      out = att @ V.T = (att_unnorm @ Wv) @ xT + att_unnorm @ bv, normalized at the end.

Precision split (validated by host simulation, rel err ~6.6e-3 vs 2e-2 gate):
  - x (both layouts), Wv, attT, bT, out: bf16  (out-path has no softmax
    error amplification; bf16 halves DMA bytes 42MB -> 22MB per core)
  - Wq/Wk/G-copies/U/S/expS: float32r (S feeds a sharp softmax; bf16 here
    pushes rel err to 1.7e-2)

Sharding: pure data parallel, batch N=16 -> 2 per core across 8 cores.
Schedule: G(0) -> mid(0) interleaved w/ G(1) -> mid(1) interleaved w/ out(0)
-> out(0) rest -> out(1). Input loads sequenced on the sync DMA queue
(xs0, weights, xs1, xt0, xt1); output writes go out on the gpsimd DMA
queue so the final pieces drain in parallel instead of queueing behind
loads. PSUM packed to exactly 8 banks: G psum 3 (upper-triangle blocks
only), mid rotation 2, stage 1, out chunks 2.
"""

import numpy as np

N, C, HW, D = 16, 512, 3136, 512
NCORES = 8
NPC = N // NCORES          # batch elements per core
CT = C // 128              # 4 c partition tiles
DT = D // 128              # 4 d partition tiles
KT = 25                    # s k-tiles: 24 x 128 + 1 x 64
KT_ROWS = [128] * 24 + [64]
OC = 448                   # out-phase s chunk
NOC = HW // OC             # 7 chunks


def _f32r_round(a: np.ndarray) -> np.ndarray:
    """Round fp32 to float32r (11 explicit mantissa bits, round-to-nearest)."""
    bits = np.ascontiguousarray(a, dtype=np.float32).view(np.uint32)
    bits = (bits + np.uint32(0x800)) & np.uint32(0xFFFFF000)
    return bits.view(np.float32)


def _build_nc():
    import concourse.mybir as mybir
    from concourse import bacc
    from concourse.tile import TileContext

    f32 = mybir.dt.float32
    f32r = mybir.dt.float32r
    bf16 = mybir.dt.bfloat16
    MUL = mybir.AluOpType.mult
    ADD = mybir.AluOpType.add
    IDENT = mybir.ActivationFunctionType.Identity
    EXP = mybir.ActivationFunctionType.Exp

    nc = bacc.Bacc("TRN2", target_bir_lowering=False, debug=False,
                   num_devices=NCORES)

    x_ext = nc.declare_dram_parameter("x", [NPC, C, HW], bf16, isOutput=False)
    xs_ext = nc.declare_dram_parameter("xs", [NPC, HW, C], bf16, isOutput=False)
    m_ext = nc.declare_dram_parameter("m", [NPC, C], f32r, isOutput=False)
    u4_ext = nc.declare_dram_parameter("u4h", [NPC, D], f32r, isOutput=False)
    wqT_ext = nc.declare_dram_parameter("wqT", [C + 1, D], f32r, isOutput=False)
    wkT_ext = nc.declare_dram_parameter("wkT", [C + 1, D], f32r, isOutput=False)
    wv_ext = nc.declare_dram_parameter("wv", [D, C], bf16, isOutput=False)
    id_ext = nc.declare_dram_parameter("ident", [128, 128], f32r, isOutput=False)
    bkb_ext = nc.declare_dram_parameter("bkb", [128, D], f32r, isOutput=False)
    bvb_ext = nc.declare_dram_parameter("bvb", [128, D], f32r, isOutput=False)
    out_ext = nc.declare_dram_parameter("out", [NPC, D, HW], bf16, isOutput=True)

    with TileContext(nc) as tc:
        with (
            tc.tile_pool(name="wpool", bufs=1) as wp,
            tc.tile_pool(name="xbig", bufs=4) as xb,
            tc.tile_pool(name="work", bufs=1) as wkp,
            tc.tile_pool(name="small", bufs=2) as sm,
            tc.tile_pool(name="outsb", bufs=6) as osb,
            tc.tile_pool(name="psum", bufs=1, space="PSUM") as ps,
        ):
            # ---- weights (loaded once) ----
            wq_t = wp.tile([128, CT, D], f32r, tag="wq")
            wq4 = wp.tile([1, D], f32r, tag="wq4")
            wk_t = wp.tile([128, CT, D], f32r, tag="wk")
            wv_t = wp.tile([128, DT, C], bf16, tag="wv")
            ident = wp.tile([128, 128], f32r, tag="ident")
            bkb = wp.tile([128, D], f32r, tag="bkb")
            bvb = wp.tile([128, D], f32r, tag="bvb")

            st = {0: {}, 1: {}}

            def load_weights_scalar():
                # U/bias-path weights on the scalar engine's DMA queue
                # (slow ring -- keep it under ~2.5MB); wq goes on sync.
                nc.scalar.dma_start(out=ident[:], in_=id_ext[:])
                nc.scalar.dma_start(out=wk_t[:],
                                    in_=wkT_ext[0:C, :].rearrange("(ci p) d -> p ci d", p=128))
                nc.scalar.dma_start(out=bkb[:], in_=bkb_ext[:])
                nc.scalar.dma_start(out=wv_t[:],
                                    in_=wv_ext[:].rearrange("(ci p) d -> p ci d", p=128))
                nc.scalar.dma_start(out=bvb[:], in_=bvb_ext[:])

            def load_wq_sync():
                nc.sync.dma_start(out=wq_t[:],
                                  in_=wqT_ext[0:C, :].rearrange("(ci p) d -> p ci d", p=128))
                nc.sync.dma_start(out=wq4[:], in_=wqT_ext[C:C + 1, :])
                for n in range(NPC):
                    m_r = sm.tile([128, CT], f32r, tag="mr", name=f"mr{n}")
                    nc.sync.dma_start(out=m_r[:],
                                      in_=m_ext[n, :].rearrange("(ci c) -> c ci", c=128))
                    u4 = sm.tile([1, D], f32r, tag="u4", name=f"u4{n}")
                    nc.sync.dma_start(out=u4[:], in_=u4_ext[n:n + 1, :])
                    st[n]["m_r"] = m_r
                    st[n]["u4"] = u4

            def phase_load_xs(n, groups, split=False):
                xs = xb.tile([128, KT, C], bf16, tag="xbig", name=f"xs{n}")
                kt0 = 0
                for gi, nkt in enumerate(groups):
                    s0, s1 = kt0 * 128, min((kt0 + nkt) * 128, HW)
                    eng = nc.gpsimd if (split and gi % 2 == 1) else nc.sync
                    if s1 - s0 == nkt * 128:
                        eng.dma_start(
                            out=xs[:, kt0:kt0 + nkt, :],
                            in_=xs_ext[n, s0:s1, :].rearrange("(k p) c -> p k c", p=128))
                    else:
                        eng.dma_start(out=xs[:s1 - s0, kt0, :],
                                      in_=xs_ext[n, s0:s1, :])
                    kt0 += nkt
                st[n]["xs"] = xs

            def phase_load_xt(n):
                xt = xb.tile([128, CT, HW], bf16, tag="xbig", name=f"xt{n}")
                for hoff, hw_ in [(0, 1568), (1568, 1568)]:
                    for ci in range(CT):
                        nc.gpsimd.dma_start(out=xt[:, ci, hoff:hoff + hw_],
                                            in_=x_ext[n, ci * 128:(ci + 1) * 128,
                                                      hoff:hoff + hw_])
                st[n]["xt"] = xt

            # PE warm-up: sustained activity flips the HAM clock gate to
            # full speed; G(0) continues the activity stream afterwards.
            warm_sb = wp.tile([128, 128], mybir.dt.bfloat16, tag="warm")
            warm_ps = ps.tile([128, 512], f32, tag="stage", name="warm_ps")
            nc.vector.memset(warm_sb[:], 0.0)
            for wi in range(24):
                nc.tensor.matmul(warm_ps[:, 0:128], warm_sb[:], warm_sb[:],
                                 start=True, stop=True)

            def phase_G(n):
                """Gram upper triangle+diag only: row-block j computes cols
                [j*128, 512); lower blocks come from transposes in mid."""
                xs = st[n]["xs"]
                gj0 = ps.tile([128, 512], f32, tag="gj0", name=f"gj0_{n}")
                gj1 = ps.tile([128, 384], f32, tag="gj1", name=f"gj1_{n}")
                gj2 = ps.tile([128, 256], f32, tag="gj2", name=f"gj2_{n}")
                # j3 diag block rides an "ops" bank: out-phase o_ps groups
                # never overlap a G phase, and concurrent accumulation
                # groups must not share a PSUM bank (start zeroes the bank).
                gj3 = ps.tile([128, 128], f32, tag="ops", bufs=2,
                              name=f"gj3_{n}")
                st[n]["g_ps"] = (gj0, gj1, gj2, gj3)
                for kt in range(KT):
                    rows = KT_ROWS[kt]
                    fl = (kt == 0, kt == KT - 1)
                    nc.tensor.matmul(gj0[:, 0:512], xs[:rows, kt, 0:128],
                                     xs[:rows, kt, 0:512], start=fl[0], stop=fl[1])
                    nc.tensor.matmul(gj1[:, 0:384], xs[:rows, kt, 128:256],
                                     xs[:rows, kt, 128:512], start=fl[0], stop=fl[1])
                    nc.tensor.matmul(gj2[:, 0:256], xs[:rows, kt, 256:384],
                                     xs[:rows, kt, 256:512], start=fl[0], stop=fl[1])
                    nc.tensor.matmul(gj3[:, 0:128], xs[:rows, kt, 384:512],
                                     xs[:rows, kt, 384:512], start=fl[0], stop=fl[1])
                    yield

            def phase_mid(n, fill=None):
                def pump(k):
                    if fill is not None:
                        for _ in range(k):
                            fill()
                gj0, gj1, gj2, gj3 = st[n]["g_ps"]
                m_r = st[n]["m_r"]
                u4 = st[n]["u4"]

                # G psum -> SBUF (computed ranges only), spread across engines
                g = wkp.tile([128, CT, 512], f32r, tag="g", name=f"g{n}")
                nc.vector.tensor_copy(g[:, 0, 0:512], gj0[:, 0:512])
                nc.scalar.activation(g[:, 1, 128:512], gj1[:, 0:384], IDENT,
                                     bias=0.0, scale=1.0)
                nc.vector.tensor_copy(g[:, 2, 256:512], gj2[:, 0:256])
                nc.vector.tensor_copy(g[:, 3, 384:512], gj3[:, 0:128])

                # lower-triangle fills via PE transposes of the upper blocks
                gfA = ps.tile([128, 512], f32r, tag="stage", name=f"gfA{n}")
                for fi, (bi, bj) in enumerate([(1, 0), (2, 0), (2, 1), (3, 0)]):
                    nc.tensor.transpose(gfA[:, fi * 128:(fi + 1) * 128],
                                        g[:, bj, bi * 128:(bi + 1) * 128], ident[:])
                nc.vector.tensor_copy(g[:, 1, 0:128], gfA[:, 0:128])
                nc.vector.tensor_copy(g[:, 2, 0:256], gfA[:, 128:384])
                nc.scalar.activation(g[:, 3, 0:128], gfA[:, 384:512], IDENT,
                                     bias=0.0, scale=1.0)

                # U = G~ @ WkT + m (x) bk ; j=3 needs no fills -> do it first
                # while gfB transposes + fill copies complete.
                u = wkp.tile([128, CT, D], f32r, tag="u", name=f"u{n}")
                first = True
                for j in [3, 2, 1, 0]:
                    u_ps = ps.tile([128, 512], f32, tag="mid", bufs=2,
                                   name=f"u_ps{n}_{j}")
                    for ki in range(CT):
                        nc.tensor.matmul(u_ps[:], g[:, ki, j * 128:(j + 1) * 128],
                                         wk_t[:, ki, :], start=(ki == 0),
                                         stop=(ki == CT - 1))
                        if first and ki == CT - 1:
                            # stage slot free again (gfA copies done during
                            # U(3)); emit remaining two fills
                            first = False
                            gfB = ps.tile([128, 256], f32r, tag="stage",
                                          name=f"gfB{n}")
                            for fi, (bi, bj) in enumerate([(3, 1), (3, 2)]):
                                nc.tensor.transpose(
                                    gfB[:, fi * 128:(fi + 1) * 128],
                                    g[:, bj, bi * 128:(bi + 1) * 128], ident[:])
                            nc.vector.tensor_copy(g[:, 3, 128:384], gfB[:, 0:256])
                    nc.vector.scalar_tensor_tensor(
                        u[:, j, :], bkb[:], m_r[:, j:j + 1], u_ps[:],
                        op0=MUL, op1=ADD)
                    pump(1)

                # S = Wq_aug @ U_aug ; softmax row stats
                expS = wkp.tile([128, DT, D], f32r, tag="expS", name=f"expS{n}")
                negmax = sm.tile([128, DT], f32, tag="negmax", name=f"negmax{n}")
                sumexp = sm.tile([128, DT], f32, tag="sumexp", name=f"sumexp{n}")
                recip = sm.tile([128, DT], f32, tag="recip", name=f"recip{n}")
                bias_d = sm.tile([128, DT], f32, tag="bias_d", name=f"bias_d{n}")
                bias_dummy = sm.tile([128, 512], f32, tag="bias_dummy",
                                     name=f"bias_dummy{n}")
                for jd in range(DT):
                    s_ps = ps.tile([128, 512], f32, tag="mid", bufs=2,
                                   name=f"s_ps{n}_{jd}")
                    for k in range(CT):
                        nc.tensor.matmul(s_ps[:], wq_t[:, k, jd * 128:(jd + 1) * 128],
                                         u[:, k, :], start=(k == 0), stop=False)
                    nc.tensor.matmul(s_ps[:], wq4[:, jd * 128:(jd + 1) * 128],
                                     u4[:], start=False, stop=True)
                    nc.vector.reduce_max(negmax[:, jd:jd + 1], s_ps[:],
                                         axis=mybir.AxisListType.X, negate=True)
                    nc.scalar.activation(expS[:, jd, :], s_ps[:], EXP,
                                         bias=negmax[:, jd:jd + 1], scale=1.0,
                                         accum_out=sumexp[:, jd:jd + 1])
                    nc.vector.scalar_tensor_tensor(
                        bias_dummy[:], expS[:, jd, :], 1.0, bvb[:],
                        op0=MUL, op1=MUL, accum_out=bias_d[:, jd:jd + 1])
                    pump(1)
                nc.vector.reciprocal(recip[:], sumexp[:])
                bias_eff = sm.tile([128, DT], f32, tag="bias_eff",
                                   name=f"bias_eff{n}")
                nc.vector.tensor_mul(bias_eff[:], bias_d[:], recip[:])

                # attT via PE transposes (bf16 store for the out path);
                # copies on the scalar engine -- the vector queue still
                # drains softmax stats and would stall the stage-slot WAR.
                attT = wkp.tile([128, DT, D], bf16, tag="attT", name=f"attT{n}")
                for je in range(DT):
                    at = ps.tile([128, 512], f32r, tag="stage", name=f"at{n}_{je}")
                    for jd in range(DT):
                        nc.tensor.transpose(at[:, jd * 128:(jd + 1) * 128],
                                            expS[:, jd, je * 128:(je + 1) * 128],
                                            ident[:])
                    nc.scalar.activation(attT[:, je, :], at[:], IDENT,
                                         bias=0.0, scale=1.0)
                    pump(2)

                # B^T = Wv^T @ attT (bf16)
                bT = wkp.tile([128, CT, D], bf16, tag="bT", bufs=2,
                              name=f"bT{n}")
                for jc in range(CT):
                    b_ps = ps.tile([128, 512], f32, tag="mid", bufs=2,
                                   name=f"b_ps{n}_{jc}")
                    for je in range(DT):
                        nc.tensor.matmul(b_ps[:], wv_t[:, je, jc * 128:(jc + 1) * 128],
                                         attT[:, je, :], start=(je == 0),
                                         stop=(je == DT - 1))
                    nc.vector.tensor_copy(bT[:, jc, :], b_ps[:])
                    pump(2)
                st[n]["bT"] = bT
                st[n]["recip"] = recip
                st[n]["bias_eff"] = bias_eff

            # o_ps banks rotate through ops x2 + the gj0/gj1 banks (idle
            # during out phases -- G accumulation groups are all closed)
            O_TAGS = [("ops", 2), ("gj0", 1), ("ops", 2), ("gj1", 1)]
            o_idx = [0]

            def phase_out(n):
                bT, recip, bias_eff = st[n]["bT"], st[n]["recip"], st[n]["bias_eff"]
                xt = st[n]["xt"]
                for jd in range(DT):
                    pieces = [(0, 1, 2), (3, 4, 5, 6)]
                    if n == NPC - 1 and jd == DT - 1:
                        pieces = [(0, 1, 2), (3, 4), (5,), (6,)]
                    for chs in pieces:
                        piece = OC * len(chs)
                        off0 = chs[0] * OC
                        o_sb = osb.tile([128, 1792], mybir.dt.bfloat16, tag="osb",
                                        name=f"o_sb{n}_{jd}_{off0}")
                        for c2, ch in enumerate(chs):
                            otag, obufs = O_TAGS[o_idx[0] % 4]
                            o_idx[0] += 1
                            o_ps = ps.tile([128, OC], f32, tag=otag, bufs=obufs,
                                           name=f"o_ps{n}_{jd}_{ch}")
                            for k in range(CT):
                                nc.tensor.matmul(o_ps[:], bT[:, k, jd * 128:(jd + 1) * 128],
                                                 xt[:, k, ch * OC:(ch + 1) * OC],
                                                 start=(k == 0), stop=(k == CT - 1))
                            if ch % 2 == 0:
                                nc.scalar.activation(o_sb[:, c2 * OC:(c2 + 1) * OC], o_ps[:],
                                                     IDENT,
                                                     bias=bias_eff[:, jd:jd + 1],
                                                     scale=recip[:, jd:jd + 1])
                            else:
                                nc.vector.tensor_scalar(o_sb[:, c2 * OC:(c2 + 1) * OC], o_ps[:],
                                                        recip[:, jd:jd + 1],
                                                        bias_eff[:, jd:jd + 1],
                                                        op0=MUL, op1=ADD)
                            yield
                        nc.gpsimd.dma_start(
                            out=out_ext[n, jd * 128:(jd + 1) * 128,
                                        off0:off0 + piece],
                            in_=o_sb[:, 0:piece])

            # ---- schedule ----
            # sync queue:   xs(0) evens, wq, m/u4, xs(1)
            # gpsimd queue: xs(0) odds, xt(0), xt(1), out writes
            # scalar queue: wk + remaining weights
            phase_load_xs(0, [1, 1, 2, 2, 2, 4, 4, 4, 4, 1], split=True)
            load_weights_scalar()
            load_wq_sync()
            phase_load_xt(0)
            phase_load_xt(1)
            phase_load_xs(1, [4, 4, 4, 4, 4, 4, 1])

            for _ in phase_G(0):
                pass
            gG1 = phase_G(1)
            phase_mid(0, fill=lambda: next(gG1, None))
            for _ in gG1:
                pass
            gO0 = phase_out(0)
            phase_mid(1, fill=lambda: next(gO0, None))
            for _ in gO0:
                pass
            for _ in phase_out(1):
                pass
    nc.compile()
    return nc


_NC_CACHE = None


def kernel(**inputs: np.ndarray) -> np.ndarray:
    global _NC_CACHE
    import ml_dtypes
    from concourse.bass_utils import run_bass_kernel_spmd

    batch = np.asarray(inputs["batch_flat"], dtype=np.float32)
    Wq = np.asarray(inputs["Wq"], dtype=np.float32)
    bq = np.asarray(inputs["bq"], dtype=np.float32)
    Wk = np.asarray(inputs["Wk"], dtype=np.float32)
    bk = np.asarray(inputs["bk"], dtype=np.float32)
    Wv = np.asarray(inputs["Wv"], dtype=np.float32)
    bv = np.asarray(inputs["bv"], dtype=np.float32)

    if _NC_CACHE is None:
        _NC_CACHE = _build_nc()
    nc = _NC_CACHE

    bf = ml_dtypes.bfloat16
    x_b = batch.astype(bf)                                    # [N, C, HW] bf16
    xs_b = np.ascontiguousarray(x_b.transpose(0, 2, 1))        # [N, HW, C] bf16
    m_r = _f32r_round(batch.astype(np.float64).sum(axis=2).astype(np.float32))
    wqT = _f32r_round(np.concatenate([Wq.T, bq[None, :]], axis=0))
    wkT = _f32r_round(np.concatenate([Wk.T, bk[None, :]], axis=0))
    u4h = _f32r_round((m_r.astype(np.float64) @ wkT[:C].astype(np.float64)
                       + HW * wkT[C].astype(np.float64)).astype(np.float32))
    wv = Wv.astype(bf)
    bkb = np.ascontiguousarray(np.tile(wkT[C:C + 1, :], (128, 1)))
    bvb = np.ascontiguousarray(np.tile(_f32r_round(bv)[None, :], (128, 1)))
    ident = np.eye(128, dtype=np.float32)

    in_maps = []
    for c in range(NCORES):
        in_maps.append({
            "x": np.ascontiguousarray(x_b[c * NPC:(c + 1) * NPC]),
            "xs": xs_b[c * NPC:(c + 1) * NPC],
            "m": m_r[c * NPC:(c + 1) * NPC],
            "u4h": u4h[c * NPC:(c + 1) * NPC],
            "wqT": wqT, "wkT": wkT, "wv": wv, "bvb": bvb, "ident": ident,
            "bkb": bkb,
        })
    r = run_bass_kernel_spmd(nc, in_maps, core_ids=list(range(NCORES)))
    out = np.concatenate([np.asarray(r.results[c]["out"])
                          for c in range(NCORES)], axis=0)
    return out.astype(np.float32)


# revision 31
# speedup vs baseline: 1.0082x; 1.0082x over previous
"""Trainium2 Bass kernel for batched channel-attention (nn_Attention_28071906246667).

Reference computation (per batch element n, with xT = batch_flat[n] of shape [C, HW]):
    x   = xT.T                                  # [HW, C]
    Q   = x @ Wq.T + bq ; K, V likewise         # [HW, D]
    S   = Q.T @ K                               # [D, D]
    att = softmax(S, axis=-1)
    out = att @ V.T                             # [D, HW]

Key algebraic restructuring (halves FLOPs, avoids materializing Q/K/V):
    G = x.T x  (Gram over channels), m = column sums of x. Then
      S   = Wq G Wk.T + (Wq m) bk.T + bq (Wk m).T + HW bq bk.T
          = Wq_aug @ U,   U = [G m; m.T HW] @ WkT_aug
      out = att @ V.T = (att_unnorm @ Wv) @ xT + att_unnorm @ bv, normalized at the end.

Precision split (validated by host simulation, rel err ~6.6e-3 vs 2e-2 gate):
  - x (both layouts), Wv, attT, bT, out: bf16  (out-path has no softmax
    error amplification; bf16 halves DMA bytes 42MB -> 22MB per core)
  - Wq/Wk/G-copies/U/S/expS: float32r (S feeds a sharp softmax; bf16 here
    pushes rel err to 1.7e-2)

Sharding: pure data parallel, batch N=16 -> 2 per core across 8 cores.
Schedule: G(0) -> mid(0) interleaved w/ G(1) -> mid(1) interleaved w/ out(0)
-> out(0) rest -> out(1). Input loads sequenced on the sync DMA queue
(xs0, weights, xs1, xt0, xt1); output writes go out on the gpsimd DMA
queue so the final pieces drain in parallel instead of queueing behind
loads. PSUM packed to exactly 8 banks: G psum 3 (upper-triangle blocks
only), mid rotation 2, stage 1, out chunks 2.
"""

import numpy as np

N, C, HW, D = 16, 512, 3136, 512
NCORES = 8
NPC = N // NCORES          # batch elements per core
CT = C // 128              # 4 c partition tiles
DT = D // 128              # 4 d partition tiles
KT = 25                    # s k-tiles: 24 x 128 + 1 x 64
KT_ROWS = [128] * 24 + [64]
OC = 448                   # out-phase s chunk
NOC = HW // OC             # 7 chunks


def _f32r_round(a: np.ndarray) -> np.ndarray:
    """Round fp32 to float32r (11 explicit mantissa bits, round-to-nearest)."""
    bits = np.ascontiguousarray(a, dtype=np.float32).view(np.uint32)
    bits = (bits + np.uint32(0x800)) & np.uint32(0xFFFFF000)
    return bits.view(np.float32)


def _build_nc():
    import concourse.mybir as mybir
    from concourse import bacc
    from concourse.tile import TileContext

    f32 = mybir.dt.float32
    f32r = mybir.dt.float32r
    bf16 = mybir.dt.bfloat16
    MUL = mybir.AluOpType.mult
    ADD = mybir.AluOpType.add
    IDENT = mybir.ActivationFunctionType.Identity
    EXP = mybir.ActivationFunctionType.Exp

    nc = bacc.Bacc("TRN2", target_bir_lowering=False, debug=False,
                   num_devices=NCORES)

    x_ext = nc.declare_dram_parameter("x", [NPC, C, HW], bf16, isOutput=False)
    xs_ext = nc.declare_dram_parameter("xs", [NPC, HW, C], bf16, isOutput=False)
    m_ext = nc.declare_dram_parameter("m", [NPC, C], f32r, isOutput=False)
    u4_ext = nc.declare_dram_parameter("u4h", [NPC, D], f32r, isOutput=False)
    wqT_ext = nc.declare_dram_parameter("wqT", [C + 1, D], f32r, isOutput=False)
    wkT_ext = nc.declare_dram_parameter("wkT", [C + 1, D], f32r, isOutput=False)
    wv_ext = nc.declare_dram_parameter("wv", [D, C], bf16, isOutput=False)
    id_ext = nc.declare_dram_parameter("ident", [128, 128], f32r, isOutput=False)
    bkb_ext = nc.declare_dram_parameter("bkb", [128, D], f32r, isOutput=False)
    bvb_ext = nc.declare_dram_parameter("bvb", [128, D], f32r, isOutput=False)
    out_ext = nc.declare_dram_parameter("out", [NPC, D, HW], bf16, isOutput=True)

    with TileContext(nc) as tc:
        with (
            tc.tile_pool(name="wpool", bufs=1) as wp,
            tc.tile_pool(name="xbig", bufs=4) as xb,
            tc.tile_pool(name="work", bufs=1) as wkp,
            tc.tile_pool(name="small", bufs=2) as sm,
            tc.tile_pool(name="outsb", bufs=6) as osb,
            tc.tile_pool(name="psum", bufs=1, space="PSUM") as ps,
        ):
            # ---- weights (loaded once) ----
            wq_t = wp.tile([128, CT, D], f32r, tag="wq")
            wq4 = wp.tile([1, D], f32r, tag="wq4")
            wk_t = wp.tile([128, CT, D], f32r, tag="wk")
            wv_t = wp.tile([128, DT, C], bf16, tag="wv")
            ident = wp.tile([128, 128], f32r, tag="ident")
            bkb = wp.tile([128, D], f32r, tag="bkb")
            bvb = wp.tile([128, D], f32r, tag="bvb")

            st = {0: {}, 1: {}}

            def load_weights_scalar():
                # U/bias-path weights on the scalar engine's DMA queue
                # (slow ring -- keep it under ~2.5MB); wq goes on sync.
                nc.scalar.dma_start(out=ident[:], in_=id_ext[:])
                nc.scalar.dma_start(out=wk_t[:],
                                    in_=wkT_ext[0:C, :].rearrange("(ci p) d -> p ci d", p=128))
                nc.scalar.dma_start(out=bkb[:], in_=bkb_ext[:])
                for n in range(NPC):
                    m_r = sm.tile([128, CT], f32r, tag="mr", name=f"mr{n}")
                    nc.scalar.dma_start(out=m_r[:],
                                        in_=m_ext[n, :].rearrange("(ci c) -> c ci", c=128))
                    u4 = sm.tile([1, D], f32r, tag="u4", name=f"u4{n}")
                    nc.scalar.dma_start(out=u4[:], in_=u4_ext[n:n + 1, :])
                    st[n]["m_r"] = m_r
                    st[n]["u4"] = u4
                nc.scalar.dma_start(out=wv_t[:],
                                    in_=wv_ext[:].rearrange("(ci p) d -> p ci d", p=128))
                nc.scalar.dma_start(out=bvb[:], in_=bvb_ext[:])

            def load_wq_sync():
                nc.sync.dma_start(out=wq_t[:],
                                  in_=wqT_ext[0:C, :].rearrange("(ci p) d -> p ci d", p=128))
                nc.sync.dma_start(out=wq4[:], in_=wqT_ext[C:C + 1, :])

            def phase_load_xs(n, groups, split=False):
                xs = xb.tile([128, KT, C], bf16, tag="xbig", name=f"xs{n}")
                kt0 = 0
                for gi, nkt in enumerate(groups):
                    s0, s1 = kt0 * 128, min((kt0 + nkt) * 128, HW)
                    eng = nc.scalar if (split and gi % 2 == 1) else nc.sync
                    if s1 - s0 == nkt * 128:
                        eng.dma_start(
                            out=xs[:, kt0:kt0 + nkt, :],
                            in_=xs_ext[n, s0:s1, :].rearrange("(k p) c -> p k c", p=128))
                    else:
                        eng.dma_start(out=xs[:s1 - s0, kt0, :],
                                      in_=xs_ext[n, s0:s1, :])
                    kt0 += nkt
                st[n]["xs"] = xs

            def phase_load_xt(n):
                xt = xb.tile([128, CT, HW], bf16, tag="xbig", name=f"xt{n}")
                for hoff, hw_ in [(0, 1568), (1568, 1568)]:
                    for ci in range(CT):
                        nc.sync.dma_start(out=xt[:, ci, hoff:hoff + hw_],
                                          in_=x_ext[n, ci * 128:(ci + 1) * 128,
                                                    hoff:hoff + hw_])
                st[n]["xt"] = xt

            # PE warm-up: sustained activity flips the HAM clock gate to
            # full speed; G(0) continues the activity stream afterwards.
            warm_sb = wp.tile([128, 128], mybir.dt.bfloat16, tag="warm")
            warm_ps = ps.tile([128, 512], f32, tag="stage", name="warm_ps")
            nc.vector.memset(warm_sb[:], 0.0)
            for wi in range(24):
                nc.tensor.matmul(warm_ps[:, 0:128], warm_sb[:], warm_sb[:],
                                 start=True, stop=True)

            def phase_G(n):
                """Gram upper triangle+diag only: row-block j computes cols
                [j*128, 512); lower blocks come from transposes in mid."""
                xs = st[n]["xs"]
                gj0 = ps.tile([128, 512], f32, tag="gj0", name=f"gj0_{n}")
                gj1 = ps.tile([128, 384], f32, tag="gj1", name=f"gj1_{n}")
                gj2 = ps.tile([128, 256], f32, tag="gj2", name=f"gj2_{n}")
                # j3 diag block rides an "ops" bank: out-phase o_ps groups
                # never overlap a G phase, and concurrent accumulation
                # groups must not share a PSUM bank (start zeroes the bank).
                gj3 = ps.tile([128, 128], f32, tag="ops", bufs=2,
                              name=f"gj3_{n}")
                st[n]["g_ps"] = (gj0, gj1, gj2, gj3)
                for kt in range(KT):
                    rows = KT_ROWS[kt]
                    fl = (kt == 0, kt == KT - 1)
                    nc.tensor.matmul(gj0[:, 0:512], xs[:rows, kt, 0:128],
                                     xs[:rows, kt, 0:512], start=fl[0], stop=fl[1])
                    nc.tensor.matmul(gj1[:, 0:384], xs[:rows, kt, 128:256],
                                     xs[:rows, kt, 128:512], start=fl[0], stop=fl[1])
                    nc.tensor.matmul(gj2[:, 0:256], xs[:rows, kt, 256:384],
                                     xs[:rows, kt, 256:512], start=fl[0], stop=fl[1])
                    nc.tensor.matmul(gj3[:, 0:128], xs[:rows, kt, 384:512],
                                     xs[:rows, kt, 384:512], start=fl[0], stop=fl[1])
                    yield

            def phase_mid(n, fill=None):
                def pump(k):
                    if fill is not None:
                        for _ in range(k):
                            fill()
                gj0, gj1, gj2, gj3 = st[n]["g_ps"]
                m_r = st[n]["m_r"]
                u4 = st[n]["u4"]

                # G psum -> SBUF (computed ranges only), spread across engines
                g = wkp.tile([128, CT, 512], f32r, tag="g", name=f"g{n}")
                nc.vector.tensor_copy(g[:, 0, 0:512], gj0[:, 0:512])
                nc.scalar.activation(g[:, 1, 128:512], gj1[:, 0:384], IDENT,
                                     bias=0.0, scale=1.0)
                nc.vector.tensor_copy(g[:, 2, 256:512], gj2[:, 0:256])
                nc.vector.tensor_copy(g[:, 3, 384:512], gj3[:, 0:128])

                # lower-triangle fills via PE transposes of the upper blocks
                gfA = ps.tile([128, 512], f32r, tag="stage", name=f"gfA{n}")
                for fi, (bi, bj) in enumerate([(1, 0), (2, 0), (2, 1), (3, 0)]):
                    nc.tensor.transpose(gfA[:, fi * 128:(fi + 1) * 128],
                                        g[:, bj, bi * 128:(bi + 1) * 128], ident[:])
                nc.vector.tensor_copy(g[:, 1, 0:128], gfA[:, 0:128])
                nc.vector.tensor_copy(g[:, 2, 0:256], gfA[:, 128:384])
                nc.scalar.activation(g[:, 3, 0:128], gfA[:, 384:512], IDENT,
                                     bias=0.0, scale=1.0)

                # U = G~ @ WkT + m (x) bk ; j=3 needs no fills -> do it first
                # while gfB transposes + fill copies complete.
                u = wkp.tile([128, CT, D], f32r, tag="u", name=f"u{n}")
                first = True
                for j in [3, 2, 1, 0]:
                    u_ps = ps.tile([128, 512], f32, tag="mid", bufs=2,
                                   name=f"u_ps{n}_{j}")
                    for ki in range(CT):
                        nc.tensor.matmul(u_ps[:], g[:, ki, j * 128:(j + 1) * 128],
                                         wk_t[:, ki, :], start=(ki == 0),
                                         stop=(ki == CT - 1))
                        if first and ki == CT - 1:
                            # stage slot free again (gfA copies done during
                            # U(3)); emit remaining two fills
                            first = False
                            gfB = ps.tile([128, 256], f32r, tag="stage",
                                          name=f"gfB{n}")
                            for fi, (bi, bj) in enumerate([(3, 1), (3, 2)]):
                                nc.tensor.transpose(
                                    gfB[:, fi * 128:(fi + 1) * 128],
                                    g[:, bj, bi * 128:(bi + 1) * 128], ident[:])
                            nc.vector.tensor_copy(g[:, 3, 128:384], gfB[:, 0:256])
                    nc.vector.scalar_tensor_tensor(
                        u[:, j, :], bkb[:], m_r[:, j:j + 1], u_ps[:],
                        op0=MUL, op1=ADD)
                    pump(1)

                # S = Wq_aug @ U_aug ; softmax row stats
                expS = wkp.tile([128, DT, D], f32r, tag="expS", name=f"expS{n}")
                negmax = sm.tile([128, DT], f32, tag="negmax", name=f"negmax{n}")
                sumexp = sm.tile([128, DT], f32, tag="sumexp", name=f"sumexp{n}")
                recip = sm.tile([128, DT], f32, tag="recip", name=f"recip{n}")
                bias_d = sm.tile([128, DT], f32, tag="bias_d", name=f"bias_d{n}")
                bias_dummy = sm.tile([128, 512], f32, tag="bias_dummy",
                                     name=f"bias_dummy{n}")
                for jd in range(DT):
                    s_ps = ps.tile([128, 512], f32, tag="mid", bufs=2,
                                   name=f"s_ps{n}_{jd}")
                    for k in range(CT):
                        nc.tensor.matmul(s_ps[:], wq_t[:, k, jd * 128:(jd + 1) * 128],
                                         u[:, k, :], start=(k == 0), stop=False)
                    nc.tensor.matmul(s_ps[:], wq4[:, jd * 128:(jd + 1) * 128],
                                     u4[:], start=False, stop=True)
                    nc.vector.reduce_max(negmax[:, jd:jd + 1], s_ps[:],
                                         axis=mybir.AxisListType.X, negate=True)
                    nc.scalar.activation(expS[:, jd, :], s_ps[:], EXP,
                                         bias=negmax[:, jd:jd + 1], scale=1.0,
                                         accum_out=sumexp[:, jd:jd + 1])
                    nc.vector.scalar_tensor_tensor(
                        bias_dummy[:], expS[:, jd, :], 1.0, bvb[:],
                        op0=MUL, op1=MUL, accum_out=bias_d[:, jd:jd + 1])
                    pump(1)
                nc.vector.reciprocal(recip[:], sumexp[:])
                bias_eff = sm.tile([128, DT], f32, tag="bias_eff",
                                   name=f"bias_eff{n}")
                nc.vector.tensor_mul(bias_eff[:], bias_d[:], recip[:])

                # attT via PE transposes (bf16 store for the out path);
                # copies on the scalar engine -- the vector queue still
                # drains softmax stats and would stall the stage-slot WAR.
                attT = wkp.tile([128, DT, D], bf16, tag="attT", name=f"attT{n}")
                for je in range(DT):
                    at = ps.tile([128, 512], f32r, tag="stage", name=f"at{n}_{je}")
                    for jd in range(DT):
                        nc.tensor.transpose(at[:, jd * 128:(jd + 1) * 128],
                                            expS[:, jd, je * 128:(je + 1) * 128],
                                            ident[:])
                    nc.scalar.activation(attT[:, je, :], at[:], IDENT,
                                         bias=0.0, scale=1.0)
                    pump(2)

                # B^T = Wv^T @ attT (bf16)
                bT = wkp.tile([128, CT, D], bf16, tag="bT", bufs=2,
                              name=f"bT{n}")
                for jc in range(CT):
                    b_ps = ps.tile([128, 512], f32, tag="mid", bufs=2,
                                   name=f"b_ps{n}_{jc}")
                    for je in range(DT):
                        nc.tensor.matmul(b_ps[:], wv_t[:, je, jc * 128:(jc + 1) * 128],
                                         attT[:, je, :], start=(je == 0),
                                         stop=(je == DT - 1))
                    nc.vector.tensor_copy(bT[:, jc, :], b_ps[:])
                    pump(2)
                st[n]["bT"] = bT
                st[n]["recip"] = recip
                st[n]["bias_eff"] = bias_eff

            # o_ps banks rotate through ops x2 + the gj0/gj1 banks (idle
            # during out phases -- G accumulation groups are all closed)
            O_TAGS = [("ops", 2), ("gj0", 1), ("ops", 2), ("gj1", 1)]
            o_idx = [0]

            def phase_out(n):
                bT, recip, bias_eff = st[n]["bT"], st[n]["recip"], st[n]["bias_eff"]
                xt = st[n]["xt"]
                for jd in range(DT):
                    pieces = [(0, 1, 2), (3, 4, 5, 6)]
                    if n == NPC - 1 and jd == DT - 1:
                        pieces = [(0, 1, 2), (3, 4), (5,), (6,)]
                    for chs in pieces:
                        piece = OC * len(chs)
                        off0 = chs[0] * OC
                        o_sb = osb.tile([128, 1792], mybir.dt.bfloat16, tag="osb",
                                        name=f"o_sb{n}_{jd}_{off0}")
                        for c2, ch in enumerate(chs):
                            otag, obufs = O_TAGS[o_idx[0] % 4]
                            o_idx[0] += 1
                            o_ps = ps.tile([128, OC], f32, tag=otag, bufs=obufs,
                                           name=f"o_ps{n}_{jd}_{ch}")
                            for k in range(CT):
                                nc.tensor.matmul(o_ps[:], bT[:, k, jd * 128:(jd + 1) * 128],
                                                 xt[:, k, ch * OC:(ch + 1) * OC],
                                                 start=(k == 0), stop=(k == CT - 1))
                            if ch % 2 == 0:
                                nc.scalar.activation(o_sb[:, c2 * OC:(c2 + 1) * OC], o_ps[:],
                                                     IDENT,
                                                     bias=bias_eff[:, jd:jd + 1],
                                                     scale=recip[:, jd:jd + 1])
                            else:
                                nc.vector.tensor_scalar(o_sb[:, c2 * OC:(c2 + 1) * OC], o_ps[:],
                                                        recip[:, jd:jd + 1],
                                                        bias_eff[:, jd:jd + 1],
                                                        op0=MUL, op1=ADD)
                            yield
                        nc.gpsimd.dma_start(
                            out=out_ext[n, jd * 128:(jd + 1) * 128,
                                        off0:off0 + piece],
                            in_=o_sb[:, 0:piece])

            # ---- schedule ----
            # sync queue: xs(0), wq, xs(1), xt(0), xt(1)
            # scalar queue: wk + small weights    gpsimd queue: out writes
            phase_load_xs(0, [1, 1, 2, 2, 2, 4, 4, 4, 4, 1])
            load_weights_scalar()
            load_wq_sync()
            phase_load_xs(1, [4, 4, 4, 4, 4, 4, 1])
            phase_load_xt(0)
            phase_load_xt(1)

            for _ in phase_G(0):
                pass
            gG1 = phase_G(1)
            phase_mid(0, fill=lambda: next(gG1, None))
            for _ in gG1:
                pass
            gO0 = phase_out(0)
            phase_mid(1, fill=lambda: next(gO0, None))
            for _ in gO0:
                pass
            for _ in phase_out(1):
                pass
    nc.compile()
    return nc


_NC_CACHE = None


def kernel(**inputs: np.ndarray) -> np.ndarray:
    global _NC_CACHE
    import ml_dtypes
    from concourse.bass_utils import run_bass_kernel_spmd

    batch = np.asarray(inputs["batch_flat"], dtype=np.float32)
    Wq = np.asarray(inputs["Wq"], dtype=np.float32)
    bq = np.asarray(inputs["bq"], dtype=np.float32)
    Wk = np.asarray(inputs["Wk"], dtype=np.float32)
    bk = np.asarray(inputs["bk"], dtype=np.float32)
    Wv = np.asarray(inputs["Wv"], dtype=np.float32)
    bv = np.asarray(inputs["bv"], dtype=np.float32)

    if _NC_CACHE is None:
        _NC_CACHE = _build_nc()
    nc = _NC_CACHE

    bf = ml_dtypes.bfloat16
    x_b = batch.astype(bf)                                    # [N, C, HW] bf16
    xs_b = np.ascontiguousarray(x_b.transpose(0, 2, 1))        # [N, HW, C] bf16
    m_r = _f32r_round(batch.astype(np.float64).sum(axis=2).astype(np.float32))
    wqT = _f32r_round(np.concatenate([Wq.T, bq[None, :]], axis=0))
    wkT = _f32r_round(np.concatenate([Wk.T, bk[None, :]], axis=0))
    u4h = _f32r_round((m_r.astype(np.float64) @ wkT[:C].astype(np.float64)
                       + HW * wkT[C].astype(np.float64)).astype(np.float32))
    wv = Wv.astype(bf)
    bkb = np.ascontiguousarray(np.tile(wkT[C:C + 1, :], (128, 1)))
    bvb = np.ascontiguousarray(np.tile(_f32r_round(bv)[None, :], (128, 1)))
    ident = np.eye(128, dtype=np.float32)

    in_maps = []
    for c in range(NCORES):
        in_maps.append({
            "x": np.ascontiguousarray(x_b[c * NPC:(c + 1) * NPC]),
            "xs": xs_b[c * NPC:(c + 1) * NPC],
            "m": m_r[c * NPC:(c + 1) * NPC],
            "u4h": u4h[c * NPC:(c + 1) * NPC],
            "wqT": wqT, "wkT": wkT, "wv": wv, "bvb": bvb, "ident": ident,
            "bkb": bkb,
        })
    r = run_bass_kernel_spmd(nc, in_maps, core_ids=list(range(NCORES)))
    out = np.concatenate([np.asarray(r.results[c]["out"])
                          for c in range(NCORES)], axis=0)
    return out.astype(np.float32)


# revision 33
# speedup vs baseline: 1.2058x; 1.1960x over previous
"""Trainium2 Bass kernel for batched channel-attention (nn_Attention_28071906246667).

Reference computation (per batch element n, with xT = batch_flat[n] of shape [C, HW]):
    x   = xT.T                                  # [HW, C]
    Q   = x @ Wq.T + bq ; K, V likewise         # [HW, D]
    S   = Q.T @ K                               # [D, D]
    att = softmax(S, axis=-1)
    out = att @ V.T                             # [D, HW]

Key algebraic restructuring (halves FLOPs, avoids materializing Q/K/V):
    G = x.T x  (Gram over channels), m = column sums of x. Then
      S   = Wq G Wk.T + (Wq m) bk.T + bq (Wk m).T + HW bq bk.T
          = Wq_aug @ U,   U = [G m; m.T HW] @ WkT_aug
      out = att @ V.T = (att_unnorm @ Wv) @ xT + att_unnorm @ bv, normalized at the end.

Precision split (validated by host simulation, rel err ~6.6e-3 vs 2e-2 gate):
  - x (both layouts), Wv, attT, bT, out: bf16  (out-path has no softmax
    error amplification; bf16 halves DMA bytes 42MB -> 22MB per core)
  - Wq/Wk/G-copies/U/S/expS: float32r (S feeds a sharp softmax; bf16 here
    pushes rel err to 1.7e-2)

Sharding: pure data parallel, batch N=16 -> 2 per core across 8 cores.
Schedule: G(0) -> mid(0) interleaved w/ G(1) -> mid(1) interleaved w/ out(0)
-> out(0) rest -> out(1). Input loads sequenced on the sync DMA queue
(xs0, weights, xs1, xt0, xt1); output writes go out on the gpsimd DMA
queue so the final pieces drain in parallel instead of queueing behind
loads. PSUM packed to exactly 8 banks: G psum 3 (upper-triangle blocks
only), mid rotation 2, stage 1, out chunks 2.
"""

import numpy as np

N, C, HW, D = 16, 512, 3136, 512
NCORES = 8
NPC = N // NCORES          # batch elements per core
CT = C // 128              # 4 c partition tiles
DT = D // 128              # 4 d partition tiles
KT = 25                    # s k-tiles: 24 x 128 + 1 x 64
KT_ROWS = [128] * 24 + [64]
OC = 448                   # out-phase s chunk
NOC = HW // OC             # 7 chunks


def _f32r_round(a: np.ndarray) -> np.ndarray:
    """Round fp32 to float32r (11 explicit mantissa bits, round-to-nearest)."""
    bits = np.ascontiguousarray(a, dtype=np.float32).view(np.uint32)
    bits = (bits + np.uint32(0x800)) & np.uint32(0xFFFFF000)
    return bits.view(np.float32)


def _build_nc():
    import concourse.mybir as mybir
    from concourse import bacc
    from concourse.tile import TileContext

    f32 = mybir.dt.float32
    f32r = mybir.dt.float32r
    bf16 = mybir.dt.bfloat16
    MUL = mybir.AluOpType.mult
    ADD = mybir.AluOpType.add
    IDENT = mybir.ActivationFunctionType.Identity
    EXP = mybir.ActivationFunctionType.Exp

    nc = bacc.Bacc("TRN2", target_bir_lowering=False, debug=False,
                   num_devices=NCORES)

    x_ext = nc.declare_dram_parameter("x", [NPC, C, HW], bf16, isOutput=False)
    xs_ext = nc.declare_dram_parameter("xs", [NPC, HW, C], bf16, isOutput=False)
    m_ext = nc.declare_dram_parameter("m", [NPC, C], f32r, isOutput=False)
    u4_ext = nc.declare_dram_parameter("u4h", [NPC, D], f32r, isOutput=False)
    wqT_ext = nc.declare_dram_parameter("wqT", [C + 1, D], f32r, isOutput=False)
    wkT_ext = nc.declare_dram_parameter("wkT", [C + 1, D], f32r, isOutput=False)
    wv_ext = nc.declare_dram_parameter("wv", [D, C], bf16, isOutput=False)
    id_ext = nc.declare_dram_parameter("ident", [128, 128], f32r, isOutput=False)
    bkb_ext = nc.declare_dram_parameter("bkb", [128, D], f32r, isOutput=False)
    bvb_ext = nc.declare_dram_parameter("bvb", [128, D], f32r, isOutput=False)
    out_ext = nc.declare_dram_parameter("out", [NPC, D, HW], bf16, isOutput=True)

    with TileContext(nc) as tc:
        with (
            tc.tile_pool(name="wpool", bufs=1) as wp,
            tc.tile_pool(name="xbig", bufs=4) as xb,
            tc.tile_pool(name="work", bufs=1) as wkp,
            tc.tile_pool(name="small", bufs=2) as sm,
            tc.tile_pool(name="outsb", bufs=6) as osb,
            tc.tile_pool(name="psum", bufs=1, space="PSUM") as ps,
        ):
            # ---- weights (loaded once) ----
            wq_t = wp.tile([128, CT, D], f32r, tag="wq")
            wq4 = wp.tile([1, D], f32r, tag="wq4")
            wk_t = wp.tile([128, CT, D], f32r, tag="wk")
            wv_t = wp.tile([128, DT, C], bf16, tag="wv")
            ident = wp.tile([128, 128], f32r, tag="ident")
            bkb = wp.tile([128, D], f32r, tag="bkb")
            bvb = wp.tile([128, D], f32r, tag="bvb")

            st = {0: {}, 1: {}}

            def load_weights_scalar():
                # U/bias-path weights on the scalar engine's DMA queue
                # (slow ring -- keep it under ~2.5MB); wq goes on sync.
                nc.scalar.dma_start(out=ident[:], in_=id_ext[:])
                nc.scalar.dma_start(out=bkb[:], in_=bkb_ext[:])
                for n in range(NPC):
                    m_r = sm.tile([128, CT], f32r, tag="mr", name=f"mr{n}")
                    nc.scalar.dma_start(out=m_r[:],
                                        in_=m_ext[n, :].rearrange("(ci c) -> c ci", c=128))
                    u4 = sm.tile([1, D], f32r, tag="u4", name=f"u4{n}")
                    nc.scalar.dma_start(out=u4[:], in_=u4_ext[n:n + 1, :])
                    st[n]["m_r"] = m_r
                    st[n]["u4"] = u4
                nc.scalar.dma_start(out=wv_t[:],
                                    in_=wv_ext[:].rearrange("(ci p) d -> p ci d", p=128))
                nc.scalar.dma_start(out=bvb[:], in_=bvb_ext[:])

            def load_wq_sync():
                nc.sync.dma_start(out=wk_t[:],
                                  in_=wkT_ext[0:C, :].rearrange("(ci p) d -> p ci d", p=128))
                nc.sync.dma_start(out=wq_t[:],
                                  in_=wqT_ext[0:C, :].rearrange("(ci p) d -> p ci d", p=128))
                nc.sync.dma_start(out=wq4[:], in_=wqT_ext[C:C + 1, :])

            def phase_load_xs(n, groups, split=False):
                xs = xb.tile([128, KT, C], bf16, tag="xbig", name=f"xs{n}")
                kt0 = 0
                for gi, nkt in enumerate(groups):
                    s0, s1 = kt0 * 128, min((kt0 + nkt) * 128, HW)
                    eng = nc.scalar if (split and gi % 2 == 1) else nc.sync
                    if s1 - s0 == nkt * 128:
                        eng.dma_start(
                            out=xs[:, kt0:kt0 + nkt, :],
                            in_=xs_ext[n, s0:s1, :].rearrange("(k p) c -> p k c", p=128))
                    else:
                        eng.dma_start(out=xs[:s1 - s0, kt0, :],
                                      in_=xs_ext[n, s0:s1, :])
                    kt0 += nkt
                st[n]["xs"] = xs

            def phase_load_xt(n):
                xt = xb.tile([128, CT, HW], bf16, tag="xbig", name=f"xt{n}")
                for hoff, hw_ in [(0, 1568), (1568, 1568)]:
                    for ci in range(CT):
                        nc.sync.dma_start(out=xt[:, ci, hoff:hoff + hw_],
                                          in_=x_ext[n, ci * 128:(ci + 1) * 128,
                                                    hoff:hoff + hw_])
                st[n]["xt"] = xt

            # PE warm-up: sustained activity flips the HAM clock gate to
            # full speed; G(0) continues the activity stream afterwards.
            warm_sb = wp.tile([128, 128], mybir.dt.bfloat16, tag="warm")
            warm_ps = ps.tile([128, 512], f32, tag="stage", name="warm_ps")
            nc.vector.memset(warm_sb[:], 0.0)
            for wi in range(16):
                nc.tensor.matmul(warm_ps[:, 0:128], warm_sb[:], warm_sb[:],
                                 start=True, stop=True)

            def phase_G(n):
                """Gram upper triangle+diag only: row-block j computes cols
                [j*128, 512); lower blocks come from transposes in mid."""
                xs = st[n]["xs"]
                gj0 = ps.tile([128, 512], f32, tag="gj0", name=f"gj0_{n}")
                gj1 = ps.tile([128, 384], f32, tag="gj1", name=f"gj1_{n}")
                gj2 = ps.tile([128, 256], f32, tag="gj2", name=f"gj2_{n}")
                # j3 diag block rides an "ops" bank: out-phase o_ps groups
                # never overlap a G phase, and concurrent accumulation
                # groups must not share a PSUM bank (start zeroes the bank).
                gj3 = ps.tile([128, 128], f32, tag="ops", bufs=2,
                              name=f"gj3_{n}")
                st[n]["g_ps"] = (gj0, gj1, gj2, gj3)
                for kt in range(KT):
                    rows = KT_ROWS[kt]
                    fl = (kt == 0, kt == KT - 1)
                    nc.tensor.matmul(gj0[:, 0:512], xs[:rows, kt, 0:128],
                                     xs[:rows, kt, 0:512], start=fl[0], stop=fl[1])
                    nc.tensor.matmul(gj1[:, 0:384], xs[:rows, kt, 128:256],
                                     xs[:rows, kt, 128:512], start=fl[0], stop=fl[1])
                    nc.tensor.matmul(gj2[:, 0:256], xs[:rows, kt, 256:384],
                                     xs[:rows, kt, 256:512], start=fl[0], stop=fl[1])
                    nc.tensor.matmul(gj3[:, 0:128], xs[:rows, kt, 384:512],
                                     xs[:rows, kt, 384:512], start=fl[0], stop=fl[1])
                    yield

            def phase_mid(n, fill=None, plan=(1, 1, 2, 2)):
                def pump(k):
                    if fill is not None:
                        for _ in range(k):
                            fill()
                gj0, gj1, gj2, gj3 = st[n]["g_ps"]
                m_r = st[n]["m_r"]
                u4 = st[n]["u4"]

                # G psum -> SBUF (computed ranges only), spread across engines
                g = wkp.tile([128, CT, 512], f32r, tag="g", name=f"g{n}")
                nc.vector.tensor_copy(g[:, 0, 0:512], gj0[:, 0:512])
                nc.scalar.activation(g[:, 1, 128:512], gj1[:, 0:384], IDENT,
                                     bias=0.0, scale=1.0)
                nc.vector.tensor_copy(g[:, 2, 256:512], gj2[:, 0:256])
                nc.vector.tensor_copy(g[:, 3, 384:512], gj3[:, 0:128])

                # lower-triangle fills via PE transposes of the upper blocks
                gfA = ps.tile([128, 512], f32r, tag="stage", name=f"gfA{n}")
                for fi, (bi, bj) in enumerate([(1, 0), (2, 0), (2, 1), (3, 0)]):
                    nc.tensor.transpose(gfA[:, fi * 128:(fi + 1) * 128],
                                        g[:, bj, bi * 128:(bi + 1) * 128], ident[:])
                nc.vector.tensor_copy(g[:, 1, 0:128], gfA[:, 0:128])
                nc.vector.tensor_copy(g[:, 2, 0:256], gfA[:, 128:384])
                nc.scalar.activation(g[:, 3, 0:128], gfA[:, 384:512], IDENT,
                                     bias=0.0, scale=1.0)

                # U = G~ @ WkT + m (x) bk ; j=3 needs no fills -> do it first
                # while gfB transposes + fill copies complete.
                u = wkp.tile([128, CT, D], f32r, tag="u", name=f"u{n}")
                first = True
                for j in [3, 2, 1, 0]:
                    u_ps = ps.tile([128, 512], f32, tag="mid", bufs=2,
                                   name=f"u_ps{n}_{j}")
                    for ki in range(CT):
                        nc.tensor.matmul(u_ps[:], g[:, ki, j * 128:(j + 1) * 128],
                                         wk_t[:, ki, :], start=(ki == 0),
                                         stop=(ki == CT - 1))
                        if first and ki == CT - 1:
                            # stage slot free again (gfA copies done during
                            # U(3)); emit remaining two fills
                            first = False
                            gfB = ps.tile([128, 256], f32r, tag="stage",
                                          name=f"gfB{n}")
                            for fi, (bi, bj) in enumerate([(3, 1), (3, 2)]):
                                nc.tensor.transpose(
                                    gfB[:, fi * 128:(fi + 1) * 128],
                                    g[:, bj, bi * 128:(bi + 1) * 128], ident[:])
                            nc.vector.tensor_copy(g[:, 3, 128:384], gfB[:, 0:256])
                    nc.vector.scalar_tensor_tensor(
                        u[:, j, :], bkb[:], m_r[:, j:j + 1], u_ps[:],
                        op0=MUL, op1=ADD)
                    pump(plan[0])

                # S = Wq_aug @ U_aug ; softmax row stats
                expS = wkp.tile([128, DT, D], f32r, tag="expS", name=f"expS{n}")
                negmax = sm.tile([128, DT], f32, tag="negmax", name=f"negmax{n}")
                sumexp = sm.tile([128, DT], f32, tag="sumexp", name=f"sumexp{n}")
                recip = sm.tile([128, DT], f32, tag="recip", name=f"recip{n}")
                bias_d = sm.tile([128, DT], f32, tag="bias_d", name=f"bias_d{n}")
                bias_dummy = sm.tile([128, 512], f32, tag="bias_dummy",
                                     name=f"bias_dummy{n}")
                for jd in range(DT):
                    s_ps = ps.tile([128, 512], f32, tag="mid", bufs=2,
                                   name=f"s_ps{n}_{jd}")
                    for k in range(CT):
                        nc.tensor.matmul(s_ps[:], wq_t[:, k, jd * 128:(jd + 1) * 128],
                                         u[:, k, :], start=(k == 0), stop=False)
                    nc.tensor.matmul(s_ps[:], wq4[:, jd * 128:(jd + 1) * 128],
                                     u4[:], start=False, stop=True)
                    nc.vector.reduce_max(negmax[:, jd:jd + 1], s_ps[:],
                                         axis=mybir.AxisListType.X, negate=True)
                    nc.scalar.activation(expS[:, jd, :], s_ps[:], EXP,
                                         bias=negmax[:, jd:jd + 1], scale=1.0,
                                         accum_out=sumexp[:, jd:jd + 1])
                    nc.vector.scalar_tensor_tensor(
                        bias_dummy[:], expS[:, jd, :], 1.0, bvb[:],
                        op0=MUL, op1=MUL, accum_out=bias_d[:, jd:jd + 1])
                    pump(plan[1])
                nc.vector.reciprocal(recip[:], sumexp[:])
                bias_eff = sm.tile([128, DT], f32, tag="bias_eff",
                                   name=f"bias_eff{n}")
                nc.vector.tensor_mul(bias_eff[:], bias_d[:], recip[:])

                # attT via PE transposes (bf16 store for the out path);
                # copies on the scalar engine -- the vector queue still
                # drains softmax stats and would stall the stage-slot WAR.
                attT = wkp.tile([128, DT, D], bf16, tag="attT", name=f"attT{n}")
                for je in range(DT):
                    at = ps.tile([128, 512], f32r, tag="stage", name=f"at{n}_{je}")
                    for jd in range(DT):
                        nc.tensor.transpose(at[:, jd * 128:(jd + 1) * 128],
                                            expS[:, jd, je * 128:(je + 1) * 128],
                                            ident[:])
                    nc.scalar.activation(attT[:, je, :], at[:], IDENT,
                                         bias=0.0, scale=1.0)
                    pump(plan[2])

                # B^T = Wv^T @ attT (bf16)
                bT = wkp.tile([128, CT, D], bf16, tag="bT", bufs=2,
                              name=f"bT{n}")
                for jc in range(CT):
                    b_ps = ps.tile([128, 512], f32, tag="mid", bufs=2,
                                   name=f"b_ps{n}_{jc}")
                    for je in range(DT):
                        nc.tensor.matmul(b_ps[:], wv_t[:, je, jc * 128:(jc + 1) * 128],
                                         attT[:, je, :], start=(je == 0),
                                         stop=(je == DT - 1))
                    nc.vector.tensor_copy(bT[:, jc, :], b_ps[:])
                    pump(plan[3])
                st[n]["bT"] = bT
                st[n]["recip"] = recip
                st[n]["bias_eff"] = bias_eff

            # o_ps banks rotate through ops x2 + the gj0/gj1 banks (idle
            # during out phases -- G accumulation groups are all closed)
            O_TAGS = [("ops", 2), ("gj0", 1), ("ops", 2), ("gj1", 1)]
            o_idx = [0]

            def phase_out(n):
                bT, recip, bias_eff = st[n]["bT"], st[n]["recip"], st[n]["bias_eff"]
                xt = st[n]["xt"]
                for jd in range(DT):
                    pieces = [(0, 1, 2), (3, 4, 5, 6)]
                    if n == NPC - 1 and jd == DT - 1:
                        pieces = [(0, 1, 2), (3, 4), (5,), (6,)]
                    for chs in pieces:
                        piece = OC * len(chs)
                        off0 = chs[0] * OC
                        o_sb = osb.tile([128, 1792], mybir.dt.bfloat16, tag="osb",
                                        name=f"o_sb{n}_{jd}_{off0}")
                        for c2, ch in enumerate(chs):
                            otag, obufs = O_TAGS[o_idx[0] % 4]
                            o_idx[0] += 1
                            o_ps = ps.tile([128, OC], f32, tag=otag, bufs=obufs,
                                           name=f"o_ps{n}_{jd}_{ch}")
                            for k in range(CT):
                                nc.tensor.matmul(o_ps[:], bT[:, k, jd * 128:(jd + 1) * 128],
                                                 xt[:, k, ch * OC:(ch + 1) * OC],
                                                 start=(k == 0), stop=(k == CT - 1))
                            if ch % 2 == 0:
                                nc.scalar.activation(o_sb[:, c2 * OC:(c2 + 1) * OC], o_ps[:],
                                                     IDENT,
                                                     bias=bias_eff[:, jd:jd + 1],
                                                     scale=recip[:, jd:jd + 1])
                            else:
                                nc.vector.tensor_scalar(o_sb[:, c2 * OC:(c2 + 1) * OC], o_ps[:],
                                                        recip[:, jd:jd + 1],
                                                        bias_eff[:, jd:jd + 1],
                                                        op0=MUL, op1=ADD)
                            yield
                        nc.gpsimd.dma_start(
                            out=out_ext[n, jd * 128:(jd + 1) * 128,
                                        off0:off0 + piece],
                            in_=o_sb[:, 0:piece])

            # ---- schedule ----
            # sync queue: xs(0), wq, xs(1), xt(0), xt(1)
            # scalar queue: wk + small weights    gpsimd queue: out writes
            phase_load_xs(0, [1, 1, 2, 2, 2, 4, 4, 4, 4, 1])
            load_weights_scalar()
            load_wq_sync()
            phase_load_xs(1, [4, 4, 4, 4, 4, 4, 1])
            phase_load_xt(0)
            phase_load_xt(1)

            for _ in phase_G(0):
                pass
            gG1 = phase_G(1)
            phase_mid(0, fill=lambda: next(gG1, None), plan=(0, 1, 2, 3))
            for _ in gG1:
                pass
            gO0 = phase_out(0)
            phase_mid(1, fill=lambda: next(gO0, None), plan=(1, 1, 2, 2))
            for _ in gO0:
                pass
            for _ in phase_out(1):
                pass
    nc.compile()
    return nc


_NC_CACHE = None


def kernel(**inputs: np.ndarray) -> np.ndarray:
    global _NC_CACHE
    import ml_dtypes
    from concourse.bass_utils import run_bass_kernel_spmd

    batch = np.asarray(inputs["batch_flat"], dtype=np.float32)
    Wq = np.asarray(inputs["Wq"], dtype=np.float32)
    bq = np.asarray(inputs["bq"], dtype=np.float32)
    Wk = np.asarray(inputs["Wk"], dtype=np.float32)
    bk = np.asarray(inputs["bk"], dtype=np.float32)
    Wv = np.asarray(inputs["Wv"], dtype=np.float32)
    bv = np.asarray(inputs["bv"], dtype=np.float32)

    if _NC_CACHE is None:
        _NC_CACHE = _build_nc()
    nc = _NC_CACHE

    bf = ml_dtypes.bfloat16
    x_b = batch.astype(bf)                                    # [N, C, HW] bf16
    xs_b = np.ascontiguousarray(x_b.transpose(0, 2, 1))        # [N, HW, C] bf16
    m_r = _f32r_round(batch.astype(np.float64).sum(axis=2).astype(np.float32))
    wqT = _f32r_round(np.concatenate([Wq.T, bq[None, :]], axis=0))
    wkT = _f32r_round(np.concatenate([Wk.T, bk[None, :]], axis=0))
    u4h = _f32r_round((m_r.astype(np.float64) @ wkT[:C].astype(np.float64)
                       + HW * wkT[C].astype(np.float64)).astype(np.float32))
    wv = Wv.astype(bf)
    bkb = np.ascontiguousarray(np.tile(wkT[C:C + 1, :], (128, 1)))
    bvb = np.ascontiguousarray(np.tile(_f32r_round(bv)[None, :], (128, 1)))
    ident = np.eye(128, dtype=np.float32)

    in_maps = []
    for c in range(NCORES):
        in_maps.append({
            "x": np.ascontiguousarray(x_b[c * NPC:(c + 1) * NPC]),
            "xs": xs_b[c * NPC:(c + 1) * NPC],
            "m": m_r[c * NPC:(c + 1) * NPC],
            "u4h": u4h[c * NPC:(c + 1) * NPC],
            "wqT": wqT, "wkT": wkT, "wv": wv, "bvb": bvb, "ident": ident,
            "bkb": bkb,
        })
    r = run_bass_kernel_spmd(nc, in_maps, core_ids=list(range(NCORES)))
    out = np.concatenate([np.asarray(r.results[c]["out"])
                          for c in range(NCORES)], axis=0)
    return out.astype(np.float32)
